# revision 36
# baseline (speedup 1.0000x reference)
# BatchGAT Trainium2 Bass kernel (v9).
#
# Reference computation (per batch b, head hd):
#   hp = h[b] @ w[hd]                      [n, 64]
#   t = tanh(hp)
#   s = t @ a_src[hd];  d = t @ a_dst[hd]  [n]
#   attn[i,j] = softmax_j(leaky_relu(s[i] + d[j], 0.2))
#   out = attn @ hp + bias_p
#
# Softmax_j is invariant to any per-i scale, so scale row i by
# exp(-0.2 s_i); additionally fold exp(d_j) into the matmul stationary
# (ed-folding):
#   hp_scaled[j, o] = hp[j, o] * exp(d_j)   (ones-column -> exp(d_j))
#   Et'[j, i] = max(exp(0.8 s_i), exp(-0.8 d_j))
#   numerator/denominator[o|1, i] = sum_j hp_scaled[j, o|1] * Et'[j, i]
# Et' is ONE single-scalar VectorE tensor_scalar_max per [128, n] tile
# (bf16 fast mode); no transcendental touches n^2 elements.  The
# weighted sums ride TensorE matmuls with hp_scaled stationary and Et'
# the N=512 moving operand, PSUM-accumulated over j-tiles.
#
# Other structure (trace-driven):
#  - h arrives pre-transposed + bf16 from the host ([2, 128, 2048]).
#  - s, d computed from tanh(hp) in [i, o] layout via DVE multiply +
#    reduce; d lands directly in column layout.  es8's row layout comes
#    from a tiny PE transpose + DRAM-roundtrip partition broadcast.
#  - bias is applied on the host; output stored f16, upcast on host.
#  - stage1(b+1) emission is interleaved into stageF(b) so the
#    broadcast roundtrip hides under main-loop matmuls; G-work of the
#    previous pair drains into the first half of the next stageF.
#
# Sharding: head-parallel, one head per NeuronCore (8 heads, 8 cores);
# each core computes all 4 batches of its head.

import numpy as np
from contextlib import ExitStack

import concourse.bass as bass
import concourse.tile as tile
import concourse.mybir as mybir
from concourse import bacc
from concourse.bass_utils import run_bass_kernel_spmd

F32 = mybir.dt.float32
BF16 = mybir.dt.bfloat16
F16 = mybir.dt.float16
AF = mybir.ActivationFunctionType
ALU = mybir.AluOpType
AX = mybir.AxisListType

NB = 4      # batches
NF = 64     # f_in == f_out
NH = 8      # heads == cores


def build_gat_module(n=2048, nb=NB):
    nc = bacc.Bacc("TRN2", target_bir_lowering=False)

    NT = n // 128          # 128-row tiles
    C512 = [(c, 512) for c in range(0, n, 512)]
    nw = len(C512)
    nhalf = nb // 2

    hT_t = nc.dram_tensor("hT", [nhalf, 128, n], BF16, kind="ExternalInput")
    w_t = nc.dram_tensor("w1", [128, NF], BF16, kind="ExternalInput")
    abc_t = nc.dram_tensor("abc", [128, 2, NF], F16, kind="ExternalInput")
    idf_t = nc.dram_tensor("idf", [128, 128], F16, kind="ExternalInput")
    idb_t = nc.dram_tensor("idb", [128, 128], BF16, kind="ExternalInput")
    o_t = nc.dram_tensor("out", [nb, n, NF], F16, kind="ExternalOutput")

    with tile.TileContext(nc) as tc:
        with ExitStack() as ctx:
            consts = ctx.enter_context(tc.tile_pool(name="consts", bufs=1))
            hpool = ctx.enter_context(tc.tile_pool(name="hpool", bufs=1))
            pairbuf = ctx.enter_context(tc.tile_pool(name="pairbuf", bufs=2))
            etp = ctx.enter_context(tc.tile_pool(name="etp", bufs=4))
            outp = ctx.enter_context(tc.tile_pool(name="outp", bufs=2))
            pst = ctx.enter_context(tc.tile_pool(name="pst", bufs=3, space="PSUM"))
            pacc = ctx.enter_context(tc.tile_pool(name="pacc", bufs=1, space="PSUM"))
            drampool = ctx.enter_context(
                tc.tile_pool(name="drampool", bufs=2, space="DRAM"))

            # ---- constants (prepped on host), warm activation tables ----
            w_sb = consts.tile([128, NF], BF16)
            nc.sync.dma_start(out=w_sb, in_=w_t[:, :])
            abc_sb = consts.tile([128, 2, NF], F16)
            nc.sync.dma_start(out=abc_sb, in_=abc_t[:, :, :])
            ident_f16 = consts.tile([128, 128], F16)
            nc.scalar.dma_start(out=ident_f16, in_=idf_t[:, :])
            ident_bf = consts.tile([128, 128], BF16)
            nc.scalar.dma_start(out=ident_bf, in_=idb_t[:, :])
            warm = consts.tile([1, 1], F32)
            nc.vector.memset(warm, 0.0)
            nc.scalar.activation(warm, warm, AF.Tanh)

            # ---- h (pre-transposed to [64, n] per batch, packed in
            # pairs on the partition axis) ----
            hT_sb = []
            for half in range(nhalf):
                t_ = hpool.tile([128, n], BF16, name=f"hT{half}")
                hT_sb.append(t_)

            def load_h(half, split):
                for icx, (c0, cs) in enumerate(C512):
                    eng = nc.scalar if (split and icx % 2 == 1) else nc.sync
                    eng.dma_start(
                        out=hT_sb[half][:, c0:c0 + cs],
                        in_=hT_t[half, :, c0:c0 + cs])

            load_h(0, split=True)

            # ---- per (batch, head-on-this-core) pair ----
            def stage1(b):
                half, bp = b // 2, NF * (b % 2)
                hT = hT_sb[half]
                w_b = w_sb[bp:bp + NF, :]
                st = {}

                # A: hp rows into PSUM (scaled copy into hp_ext below,
                # once ed is known)
                hp_ext = pairbuf.tile([128, NT, 66], BF16, name="hp_ext")
                t_sb = pairbuf.tile([128, NT, NF], F16, name="t_sb")
                psAs = []
                for grp in range(2):
                    j0 = grp * 8
                    psA = pst.tile([128, 8, NF], F32, tag="ps", name="psA")
                    for k in range(8):
                        jb = j0 + k
                        nc.tensor.matmul(
                            psA[:, k, :],
                            lhsT=hT[bp:bp + NF, jb * 128:(jb + 1) * 128],
                            rhs=w_b, start=True, stop=True)
                    nc.scalar.activation(t_sb[:, j0:j0 + 8, :], psA, AF.Tanh)
                    psAs.append(psA)
                st["hp_ext"] = hp_ext

                # s, d: DVE multiply + reduce per group, from t in [i, o]
                # layout.  d lands directly in column layout [128, NT].
                prod = pairbuf.tile([128, NT, NF], F16, name="prod")
                s_col = pairbuf.tile([128, NT], F32, name="s_col")
                d_col = pairbuf.tile([128, NT], F32, name="d_col")
                prod2 = pairbuf.tile([128, NT, NF], F16, name="prod2")
                for grp in range(2):
                    j0 = grp * 8
                    for which, col, pr in ((0, s_col, prod),
                                           (1, d_col, prod2)):
                        aap = abc_sb[:, which, :]
                        a_b = bass.AP(
                            tensor=aap.tensor, offset=aap.offset,
                            ap=[list(aap.ap[0]), [0, 8], list(aap.ap[1])])
                        nc.vector.scalar_tensor_tensor(
                            out=pr[:, j0:j0 + 8, :],
                            in0=t_sb[:, j0:j0 + 8, :], scalar=1.0, in1=a_b,
                            op0=ALU.bypass, op1=ALU.mult)
                        nc.vector.tensor_reduce(
                            out=col[:, j0:j0 + 8], in_=pr[:, j0:j0 + 8, :],
                            axis=AX.X, op=ALU.add)

                # es8 row: exp(0.8 s) in col layout, tiny PE transpose to
                # row layout, then DRAM roundtrip for partition broadcast.
                es8_col = pairbuf.tile([128, NT], BF16, name="es8_col")
                nc.scalar.activation(es8_col, s_col, AF.Exp, scale=0.8)
                psT = pst.tile([NT, 128], BF16, tag="ps", name="psT")
                nc.tensor.transpose(psT, es8_col, ident_bf)
                es8_row = pairbuf.tile([NT, 128], BF16, name="es8_row")
                nc.scalar.copy(es8_row, psT)
                es8_dram = drampool.tile([1, n], BF16, name="es8_dram")
                edap = es8_dram[0, :]
                nc.scalar.dma_start(
                    out=bass.AP(tensor=edap.tensor, offset=edap.offset,
                                ap=[[128, NT], [1, 128]]),
                    in_=es8_row)
                es8_bc = pairbuf.tile([128, n], BF16, name="es8_bc")
                for icx, (c0, cs) in enumerate(C512):
                    eng = nc.sync if icx % 2 == 0 else nc.scalar
                    eng.dma_start(
                        out=es8_bc[:, c0:c0 + cs],
                        in_=bass.AP(tensor=edap.tensor,
                                    offset=edap.offset + c0,
                                    ap=[[0, 128], [1, cs]]))
                st["es8_bc"] = es8_bc

                # ed (stationary scale), r = ed2/ed = exp(-0.8 d)
                ed_col = pairbuf.tile([128, NT], F32, name="ed_col")
                r_col = pairbuf.tile([128, NT], F32, name="r_col")
                nc.scalar.activation(r_col, d_col, AF.Exp, scale=-0.8)
                nc.scalar.activation(ed_col, d_col, AF.Exp)
                st["r_col"] = r_col

                # scaled stationary: hp_ext[:, jb, o] = hp * ed_j; the
                # denominator ones-column becomes ed_j itself.  Split the
                # 16 scale-copies across ACT and DVE to halve the chain.
                for grp in range(2):
                    j0 = grp * 8
                    for k in range(8):
                        jb = j0 + k
                        if k % 2 == 0:
                            nc.scalar.mul(
                                hp_ext[:, jb, 0:NF], psAs[grp][:, k, :],
                                ed_col[:, jb:jb + 1])
                        else:
                            nc.vector.tensor_scalar_mul(
                                hp_ext[:, jb, 0:NF], psAs[grp][:, k, :],
                                ed_col[:, jb:jb + 1])
                nc.scalar.copy(hp_ext[:, :, 64], ed_col)
                return st

            def stageG1(st):
                # drain accumulator: 65 x n f32 -> f16 with 2^-8 scale
                # (cancels in the division), split across ACT/DVE.
                accT = st["accT"]
                accT_sb = pairbuf.tile([65, n], F16, name="accT_sb")
                for icx, (c0, cs) in enumerate(C512):
                    dst = accT_sb[:, c0:c0 + cs]
                    src = accT[:, icx, 0:cs]
                    if icx % 2 == 0:
                        nc.scalar.mul(dst, src, 1.0 / 256.0)
                    else:
                        nc.vector.tensor_scalar_mul(dst, src, 1.0 / 256.0)
                st["accT_sb"] = accT_sb

            def stageF(st, b, interleave=None):
                # main loop: Et' tile via one single-scalar max op, then
                # accT[o|sum, i] += hp_scaled[jb].T @ Et'[jb], stationary
                # hp, one psum bank per 512-col chunk.
                accT = pacc.tile([65, nw, 512], F32, name="accT")
                for jb in range(NT):
                    et = etp.tile([128, n], BF16, name="et")
                    nc.vector.tensor_scalar_max(
                        et, st["es8_bc"], st["r_col"][:, jb:jb + 1])
                    for icx, (c0, cs) in enumerate(C512):
                        nc.tensor.matmul(
                            accT[:, icx, 0:cs],
                            lhsT=st["hp_ext"][:, jb, 0:65],
                            rhs=et[:, c0:c0 + cs],
                            start=(jb == 0), stop=(jb == NT - 1))
                    if interleave is not None and jb in interleave:
                        interleave[jb]()
                st["accT"] = accT
                return st

            def stageG2(st, b):
                # transpose numerators back to [i, o] on PE (fp16), then
                # reciprocal (DVE) + scale-multiply (ACT); bias is applied
                # on the host.  f16 stores split across two queues.
                accT_sb = st["accT_sb"]
                o_full = outp.tile([128, NT, NF], F16, name="o_full")
                for ic in range(NT):
                    trp = pst.tile([128, 65], F16, tag="ps", name="trp")
                    nc.tensor.transpose(
                        trp, accT_sb[:, ic * 128:(ic + 1) * 128],
                        ident_f16[0:65, 0:65])
                    r = outp.tile([128, 1], F32, name="r")
                    nc.vector.reciprocal(r, trp[:, 64:65])
                    if ic % 2 == 0:
                        nc.scalar.mul(o_full[:, ic, :], trp[:, 0:NF], r)
                    else:
                        nc.vector.tensor_scalar_mul(
                            o_full[:, ic, :], trp[:, 0:NF], r)
                oap = o_t[b, :, :]
                for icx, (c0, cs) in enumerate(C512):
                    i0 = c0 // 128
                    eng = nc.sync
                    eng.dma_start(
                        out=bass.AP(tensor=oap.tensor,
                                    offset=oap.offset + c0 * NF,
                                    ap=[[NF, 128], [128 * NF, cs // 128],
                                        [1, NF]]),
                        in_=o_full[:, i0:i0 + cs // 128, :])

            def stageG_last(st, b):
                # final pair: pipeline drain -> transpose -> divide per
                # 512-chunk so the kernel tail is as short as possible.
                accT = st["accT"]
                accT_sb = pairbuf.tile([65, n], F16, name="accT_sb")
                o_full = outp.tile([128, NT, NF], F16, name="o_full")
                oap = o_t[b, :, :]
                for icx, (c0, cs) in enumerate(C512):
                    dst = accT_sb[:, c0:c0 + cs]
                    src = accT[:, icx, 0:cs]
                    if icx % 2 == 0:
                        nc.scalar.mul(dst, src, 1.0 / 256.0)
                    else:
                        nc.vector.tensor_scalar_mul(dst, src, 1.0 / 256.0)
                    for k in range(4):
                        ic = icx * 4 + k
                        trp = pst.tile([128, 65], F16, tag="ps", name="trp")
                        nc.tensor.transpose(
                            trp, accT_sb[:, ic * 128:(ic + 1) * 128],
                            ident_f16[0:65, 0:65])
                        r = outp.tile([128, 1], F32, name="r")
                        nc.vector.reciprocal(r, trp[:, 64:65])
                        if ic % 2 == 0:
                            nc.scalar.mul(o_full[:, ic, :], trp[:, 0:NF], r)
                        else:
                            nc.vector.tensor_scalar_mul(
                                o_full[:, ic, :], trp[:, 0:NF], r)
                    i0 = c0 // 128
                    eng = nc.sync
                    eng.dma_start(
                        out=bass.AP(tensor=oap.tensor,
                                    offset=oap.offset + c0 * NF,
                                    ap=[[NF, 128], [128 * NF, cs // 128],
                                        [1, NF]]),
                        in_=o_full[:, i0:i0 + cs // 128, :])

            # ---- emission schedule ----
            sts = {}
            sts[0] = stage1(0)
            for half in range(1, nhalf):
                load_h(half, split=False)
            for b in range(nb):
                if b > 0:
                    stageG1(sts[b - 1])
                inter = {}
                if b + 1 < nb:
                    inter[3] = lambda bb=b + 1: sts.__setitem__(
                        bb, stage1(bb))
                if b > 0:
                    inter[8] = lambda bb=b - 1: stageG2(sts[bb], bb)
                stageF(sts[b], b, interleave=inter)
            stageG_last(sts[nb - 1], nb - 1)

    nc.compile()
    return nc


_CACHE = {}
_last_results = None


def _get_nc(n=2048, nb=NB):
    key = (n, nb)
    if key not in _CACHE:
        _CACHE[key] = build_gat_module(n, nb)
    return _CACHE[key]


def kernel(h, adj, w, a_src, a_dst, bias_p):
    global _last_results
    h = np.asarray(h, dtype=np.float32)
    w = np.asarray(w, dtype=np.float32)
    a_src = np.asarray(a_src, dtype=np.float32)
    a_dst = np.asarray(a_dst, dtype=np.float32)
    bias_p = np.asarray(bias_p, dtype=np.float32)
    nb, n, _ = h.shape

    nc = _get_nc(n, nb)

    # host-side prep shared by all cores
    import ml_dtypes
    hT = np.ascontiguousarray(
        h.transpose(0, 2, 1).reshape(nb // 2, 128, n)).astype(
            ml_dtypes.bfloat16)
    idf = np.eye(128, dtype=np.float16)
    idb = np.eye(128, dtype=ml_dtypes.bfloat16)

    in_maps = []
    for c in range(NH):
        wrep = np.ascontiguousarray(
            np.concatenate([w[c], w[c]], axis=0)).astype(ml_dtypes.bfloat16)
        abc = np.ascontiguousarray(np.stack(
            [np.broadcast_to(a_src[c, :, 0], (128, NF)),
             np.broadcast_to(a_dst[c, :, 0], (128, NF))],
            axis=1)).astype(np.float16)
        in_maps.append({
            "hT": hT,
            "w1": wrep,
            "abc": abc,
            "idf": idf,
            "idb": idb,
        })
    res = run_bass_kernel_spmd(nc, in_maps, core_ids=list(range(NH)))
    _last_results = res
    out = np.empty((nb, NH, n, NF), np.float32)
    for c in range(NH):
        out[:, c] = np.asarray(res.results[c]["out"]).astype(np.float32)
    out += bias_p
    return out
